# revision 3
# baseline (speedup 1.0000x reference)
"""Trainium2 Bass kernel V3 for nested triangle multiplicative update.

Data-parallel over nodes N=1024 across 8 cores (128 nodes/core). Per-core:
- rbf exponent via DMA partition-broadcast of D + ACT Square + ACT Exp
  (no PE polynomial, no fp32 matmul).
- d3 computed transposed per node: [128 pair-partitions, z] via matmuls with
  the rbf block as stationary and W_dp as moving.
- triangle j-contraction done on the PE: shifted-selector stationary matmuls
  accumulate 4 nodes x 8 blocks (+ the b_dp*sum_j(ee) bias rank-1 term) into
  one [128 edge, 128 z] PSUM tile, replacing the DVE TensorReduce.
- elementwise d3*ee runs split across DVE and GpSimd.
- LN over z happens in [edge, z] orientation (free-dim stats).
"""
import sys

sys.path.insert(0, "/opt/trn_rl_repo")

import numpy as np

import concourse.bacc as bacc
import concourse.bass as bass
import concourse.mybir as mybir
import concourse.tile as tile
from concourse.bass import AP
from concourse.bass_utils import run_bass_kernel_spmd

F32 = mybir.dt.float32
BF16 = mybir.dt.bfloat16
I16 = mybir.dt.int16

N, K, C_S, C_Z, C_G, R = 1024, 32, 384, 128, 16, 64
NCORES = 8
NN = N // NCORES          # nodes per core = 128
NK = NN * K               # edges per core = 4096
NCH = NK // 128           # 128-row chunks of edges = 32
D_MAX, EPS_LN = 20.0, 1e-5
SIGMA = D_MAX / R                     # 0.3125
MU = np.linspace(0.0, D_MAX, R)      # spacing 20/63

# fraction of q-iterations whose TT runs on gpsimd (tuned from timeline)
POOL_EVERY = 2   # n % POOL_EVERY == 1 -> gpsimd
ERF = False      # single-pass rbf via Derivative_Erf (else Square+Exp)


def ts(i, n):
    return slice(i * n, (i + 1) * n)


def bc(ap, pos, rep):
    """Insert a broadcast (step-0) dim of length rep at free-dim position pos."""
    newap = list(ap.ap)
    newap.insert(pos, [0, rep])
    return AP(ap.tensor, ap.offset, newap)


def build_nc():
    nc = bacc.Bacc("TRN2", target_bir_lowering=False, debug=False)
    P = lambda name, shape, dt: nc.declare_dram_parameter(name, list(shape), dt, isOutput=False)

    ef_d = P("ef", [NK, C_Z], F32)
    dstw_d = P("dstw", [128, K], I16)
    ntabT_d = P("ntabT", [4, N], F32)
    nfTa_d = P("nfTa", [C_S + 1, NN], F32)
    wnlra_d = P("wnlra", [C_S + 1, 2 * C_G], F32)
    wdg_d = P("wdg", [C_G * C_G, C_Z], F32)
    bdg_d = P("bdg", [C_Z], F32)
    negmu_d = P("negmu", [128], F32)
    wdp2_d = P("wdp2", [128, C_Z], BF16)
    bdp_d = P("bdp", [C_Z], F32)
    wgeg_d = P("wgeg", [C_Z, C_Z], BF16)
    wgep_d = P("wgep", [C_Z, C_Z], BF16)
    wgog_d = P("wgog", [C_Z, C_Z], BF16)
    beg_d = P("beg", [C_Z], F32)
    bep_d = P("bep", [C_Z], F32)
    bog_d = P("bog", [C_Z], F32)
    wglo_d = P("wglo", [C_Z, C_Z], BF16)
    blo2_d = P("blo2", [C_Z], F32)
    idf_d = P("idf", [128, 128], F32)
    idb_d = P("idb", [128, 128], BF16)
    selbig_d = P("selbig", [128, 252], BF16)
    sel4_d = P("sel4", [4, 128], BF16)
    out_d = nc.declare_dram_parameter("out", [NK, C_Z], F32, isOutput=True)

    mult, add_op, sub_op = (mybir.AluOpType.mult, mybir.AluOpType.add,
                            mybir.AluOpType.subtract)
    AF = mybir.ActivationFunctionType
    AX = mybir.AxisListType

    with tile.TileContext(nc) as tc:
        with (
            tc.tile_pool(name="big", bufs=1) as big,
            tc.tile_pool(name="wk", bufs=2) as wk,
            tc.tile_pool(name="wk3", bufs=3) as wk3,
            tc.tile_pool(name="wkp", bufs=5) as wkp,
            tc.tile_pool(name="wk1", bufs=1) as wk1,
            tc.tile_pool(name="ps", bufs=2, space="PSUM") as ps,
            tc.tile_pool(name="ps_d3", bufs=4, space="PSUM") as ps_d3,
            tc.tile_pool(name="ps_u", bufs=2, space="PSUM") as ps_u,
        ):
            # ---------- persistent tiles + loads ----------
            ef_t = big.tile([128, NCH, C_Z], F32)
            nc.scalar.dma_start(
                ef_t[:, :, :],
                AP(ef_d.ap().tensor, 0, [[C_Z, 128], [128 * C_Z, NCH], [1, C_Z]]),
            )
            dstw_t = big.tile([128, K], I16)
            nc.sync.dma_start(dstw_t[:], dstw_d.ap()[:, :])
            tabrep_t = big.tile([128, N, 1], F32)
            nc.sync.dma_start(
                tabrep_t[:, :, 0],
                AP(ntabT_d.ap().tensor, 0, [[0, 32], [N, 4], [1, N]]),
            )
            nfTa_t = big.tile([128, 4, NN], F32)
            for c in range(3):
                nc.sync.dma_start(nfTa_t[:, c, :], nfTa_d.ap()[ts(c, 128), :])
            nc.sync.dma_start(nfTa_t[0:1, 3, :], nfTa_d.ap()[384:385, :])
            wnlra_t = big.tile([128, 4, 2 * C_G], F32)
            for c in range(3):
                nc.sync.dma_start(wnlra_t[:, c, :], wnlra_d.ap()[ts(c, 128), :])
            nc.sync.dma_start(wnlra_t[0:1, 3, :], wnlra_d.ap()[384:385, :])
            wdg_t = big.tile([128, 2, C_Z], F32)
            nc.sync.dma_start(wdg_t[:, 0, :], wdg_d.ap()[0:128, :])
            nc.sync.dma_start(wdg_t[:, 1, :], wdg_d.ap()[128:256, :])
            wdp2_t = big.tile([128, C_Z], BF16)
            nc.scalar.dma_start(wdp2_t[:], wdp2_d.ap()[:, :])
            wgeg_t = big.tile([C_Z, C_Z], BF16)
            nc.scalar.dma_start(wgeg_t[:], wgeg_d.ap()[:, :])
            wgep_t = big.tile([C_Z, C_Z], BF16)
            nc.scalar.dma_start(wgep_t[:], wgep_d.ap()[:, :])
            wgog_t = big.tile([C_Z, C_Z], BF16)
            nc.scalar.dma_start(wgog_t[:], wgog_d.ap()[:, :])
            wglo_t = big.tile([C_Z, C_Z], BF16)
            nc.scalar.dma_start(wglo_t[:], wglo_d.ap()[:, :])
            idf_t = big.tile([128, 128], F32)
            nc.scalar.dma_start(idf_t[:], idf_d.ap()[:, :])
            idb_t = big.tile([128, 128], BF16)
            nc.scalar.dma_start(idb_t[:], idb_d.ap()[:, :])
            selbig_t = big.tile([128, 252], BF16)
            nc.scalar.dma_start(selbig_t[:], selbig_d.ap()[:, :])
            sel4_t = big.tile([4, 128], BF16)
            nc.scalar.dma_start(sel4_t[:], sel4_d.ap()[:, :])

            def colvec(name_d):
                t = big.tile([128, 1], F32, tag=f"cv_{name_d.name}")
                nc.scalar.dma_start(t[:, 0:1], AP(name_d.ap().tensor, 0, [[1, 128], [1, 1]]))
                return t
            bdg_t, negmu_t, bdp_t = colvec(bdg_d), colvec(negmu_d), colvec(bdp_d)
            beg_t, bep_t, bog_t, blo2_t = colvec(beg_d), colvec(bep_d), colvec(bog_d), colvec(blo2_d)

            # ---------- S1: gather translations ----------
            gat_t = big.tile([128, 512, 1], F32)
            nc.gpsimd.ap_gather(gat_t[:, :, :], tabrep_t[:, :, :], dstw_t[:, :],
                                channels=128, num_elems=N, d=1, num_idxs=512)
            tn_t = big.tile([128, 4, K], F32)
            for g in range(8):
                for c in range(3):
                    srcg = gat_t[16 * g + c:16 * g + c + 1, :, 0].rearrange(
                        "p (a b) -> p a b", a=16)
                    nc.sync.dma_start(tn_t[16 * g:16 * (g + 1), c, :], srcg)

            # ---------- S2: distances D (two halves of i to bound SBUF) ----------
            dD_t = big.tile([128, K * K], F32)
            for hh in range(4):
                isl = slice(hh * 8, (hh + 1) * 8)
                dif_t = wk.tile([128, 3, 8, K], F32, tag="dif")
                in_j = bc(tn_t[:, 0:3, :], 2, 8)       # [p, c, i(bc8), j]
                in_i = bc(tn_t[:, 0:3, isl], 3, K)     # [p, c, i8, j(bc)]
                nc.vector.tensor_tensor(dif_t[:, :, :, :], in_j, in_i, op=sub_op)
                d3k = dif_t[:, :, :, :].rearrange("p a b c -> p (a b c)")
                nc.vector.tensor_tensor(d3k, d3k, d3k, op=mult)
                d2c_t = wk.tile([128, 8 * K], F32, tag="d2c")
                difap = dif_t[:, :, :, :]
                nc.vector.tensor_reduce(
                    d2c_t[:, :],
                    AP(difap.tensor, difap.offset,
                       [difap.ap[0], [1, 8 * K], [8 * K, 3]]),
                    axis=AX.X, op=add_op)
                nc.vector.tensor_scalar_max(d2c_t[:, :], d2c_t[:, :], 0.0)
                nc.scalar.sqrt(dD_t[:, ts(hh, 256)], d2c_t[:, :])

            # ---------- S4: nl/nr -> outer -> gate3 (g3 stays [z, n]) ----------
            nlnr_ps = ps.tile([128, 2 * C_G], F32, tag="m")
            for c in range(4):
                kdim = 128 if c < 3 else 1
                nc.tensor.matmul(nlnr_ps[:, :], nfTa_t[0:kdim, c, :], wnlra_t[0:kdim, c, :],
                                 start=(c == 0), stop=(c == 3))
            nlnr_t = wk.tile([128, 2 * C_G], F32, tag="nlnr")
            nc.vector.tensor_copy(nlnr_t[:, :], nlnr_ps[:, :])
            outer_t = wk.tile([128, C_G * C_G], F32, tag="outer")
            for a in range(C_G):
                nc.vector.tensor_scalar_mul(outer_t[:, ts(a, C_G)], nlnr_t[:, C_G:2 * C_G],
                                            nlnr_t[:, a:a + 1])
            oT_sb = wk.tile([128, 2, 128], F32, tag="oT")
            for h in range(2):
                oT_ps = ps.tile([128, 128], F32, tag="m")
                nc.tensor.transpose(oT_ps[:, :], outer_t[:, ts(h, 128)], idf_t[:, :])
                nc.vector.tensor_copy(oT_sb[:, h, :], oT_ps[:, :])
            g3_ps = ps.tile([128, 128], F32, tag="m")
            for h in range(2):
                nc.tensor.matmul(g3_ps[:, :], wdg_t[:, h, :], oT_sb[:, h, :],
                                 start=(h == 0), stop=(h == 1))
            g3_t = wk.tile([128, NN], BF16, tag="g3")
            nc.scalar.activation(g3_t[:, :], g3_ps[:, :], AF.Sigmoid, bias=bdg_t[:, :])

            # ---------- S5: LN(ef) stats in [e, z]; xhat -> transpose -> xT ----------
            ms_t = wk.tile([128, NCH], F32, tag="ms")
            nc.vector.tensor_reduce(ms_t[:, :], ef_t[:, :, :], axis=AX.X, op=add_op)
            ss_t = wk.tile([128, NCH], F32, tag="ss")
            for c8 in range(NCH // 8):
                sqg = wk.tile([128, 8, C_Z], BF16, tag="sqg")
                nc.vector.tensor_tensor(sqg[:, :, :], ef_t[:, ts(c8, 8), :],
                                        ef_t[:, ts(c8, 8), :], op=mult)
                nc.vector.tensor_reduce(ss_t[:, ts(c8, 8)], sqg[:, :, :],
                                        axis=AX.X, op=add_op)
            mm_t = wk.tile([128, NCH], F32, tag="lnst")
            nc.vector.tensor_scalar_mul(mm_t[:, :], ms_t[:, :], 1.0 / C_Z)
            ex2_t = wk.tile([128, NCH], F32, tag="lnst2")
            nc.vector.tensor_scalar_mul(ex2_t[:, :], ss_t[:, :], 1.0 / C_Z)
            var_t = wk.tile([128, NCH], F32, tag="lnst3")
            nc.vector.tensor_tensor(var_t[:, :], mm_t[:, :], mm_t[:, :], op=mult)
            nc.vector.tensor_tensor(var_t[:, :], ex2_t[:, :], var_t[:, :], op=sub_op)
            nc.vector.tensor_scalar_add(var_t[:, :], var_t[:, :], EPS_LN)
            inv_t = wk.tile([128, NCH], F32, tag="lnst4")
            nc.vector.reciprocal(inv_t[:, :], var_t[:, :])
            rstd_t = wk.tile([128, NCH], F32, tag="lnst5")
            nc.scalar.sqrt(rstd_t[:, :], inv_t[:, :])
            mrs_t = wk.tile([128, NCH], F32, tag="lnst6")
            nc.vector.tensor_tensor(mrs_t[:, :], mm_t[:, :], rstd_t[:, :], op=mult)

            xT_t = big.tile([C_Z, NK], BF16)
            sge_t = big.tile([C_Z, NK], BF16)
            pep_t = big.tile([C_Z, NK], BF16)
            ogs_t = big.tile([C_Z, NK], BF16)
            ee_t = big.tile([C_Z, NK], BF16)
            eeT_t = big.tile([128, NCH, C_Z], BF16)
            see_t = wk1.tile([128, NN], F32, tag="see")
            sbZ_t = wk1.tile([128, NN], BF16, tag="sbZ")
            bias_stage = {}
            ee4_blk = [big.tile([128, NN // 4, C_Z], BF16, tag=f"ee4_{hb}", name=f"ee4_{hb}")
                       for hb in range(4)]

            for g4 in range(8):
                # xhat for 4 chunks -> transpose -> xT block
                xp_ps = ps.tile([128, 4, 128], BF16, tag="m", name=f"xp_{g4}")
                for j in range(4):
                    c = 4 * g4 + j
                    xh = wk.tile([128, C_Z], BF16, tag="xh", name=f"xh_{c}")
                    nc.vector.tensor_scalar(xh[:, :], ef_t[:, c, :],
                                            rstd_t[:, c:c + 1], mrs_t[:, c:c + 1],
                                            op0=mult, op1=sub_op)
                    nc.tensor.transpose(xp_ps[:, j, :], xh[:, :], idb_t[:, :])
                nc.scalar.copy(xT_t[:, ts(g4, 512)],
                               xp_ps[:, :, :].rearrange("p a b -> p (a b)"))
                # e2 gates for this 512-edge block
                for (w_t, b_t, fn, dst) in ((wgeg_t, beg_t, AF.Sigmoid, sge_t),
                                            (wgep_t, bep_t, AF.Identity, pep_t),
                                            (wgog_t, bog_t, AF.Sigmoid, ogs_t)):
                    e_ps = ps.tile([128, 512], F32, tag="m", name=f"g_{g4}_{fn}")
                    nc.tensor.matmul(e_ps[:, :], w_t[:, :], xT_t[:, ts(g4, 512)],
                                     start=True, stop=True)
                    nc.scalar.activation(dst[:, ts(g4, 512)], e_ps[:, :], fn, bias=b_t[:, :])
                nc.vector.tensor_tensor(ee_t[:, ts(g4, 512)], sge_t[:, ts(g4, 512)],
                                        pep_t[:, ts(g4, 512)], op=mult)
                # fold g3 for nodes [16*g4, 16*g4+16); see partial
                eeap = ee_t[:, :]
                ee3g = AP(eeap.tensor, eeap.offset + 512 * g4, [eeap.ap[0], [K, 16], [1, K]])
                g3bg = bc(g3_t[:, 16 * g4:16 * g4 + 16], 2, K)
                nc.vector.tensor_tensor(ee3g, ee3g, g3bg, op=mult)
                nc.vector.tensor_reduce(see_t[:, 16 * g4:16 * g4 + 16], ee3g,
                                        axis=AX.X, op=add_op)
                nc.vector.tensor_scalar_mul(sbZ_t[:, 16 * g4:16 * g4 + 16],
                                            see_t[:, 16 * g4:16 * g4 + 16], bdp_t[:, :])
                for cc in range(4 * g4, 4 * g4 + 4):
                    sb_ps = ps.tile([4, 128], BF16, tag="m", name=f"sbps_{cc}")
                    nc.tensor.transpose(sb_ps[:, :], sbZ_t[:, 4 * cc:4 * cc + 4],
                                        idb_t[:, :])
                    bst = wk1.tile([4, 128], BF16, tag=f"bst_{cc}", name=f"bst_{cc}")
                    nc.scalar.copy(bst[:, :], sb_ps[:, :])
                    bias_stage[cc] = bst
                # eeT for these 4 chunks
                ep_ps = ps.tile([128, 4, 128], BF16, tag="m", name=f"ep_{g4}")
                for j in range(4):
                    c = 4 * g4 + j
                    nc.tensor.transpose(ep_ps[:, j, :], ee_t[:, ts(c, 128)], idb_t[:, :])
                nc.scalar.copy(eeT_t[:, ts(g4, 4), :].rearrange("p a b -> p (a b)"),
                               ep_ps[:, :, :].rearrange("p a b -> p (a b)"))
                # replicate into ee4 once each 64-node half is ready
                if g4 in (1, 3, 5, 7):
                    hb = g4 // 2
                    for ip in range(4):
                        for n4 in range(4):
                            base = ee4_blk[hb][ts(ip, 32), :, :]
                            dst = AP(base.tensor, base.offset + n4 * C_Z,
                                     [list(base.ap[0]), [4 * C_Z, 8], [1, C_Z]])
                            srce = eeT_t[32 * n4:32 * n4 + 32, 8 * hb:8 * hb + 8, :]
                            nc.sync.dma_start(dst, srce)

            def ee4_node(n):
                return ee4_blk[n // 32][:, n % 32, :]

            # ---------- S7: rbf -> d3T -> prod -> PE-reduce (half-node pipeline) ----------
            upd_sb = big.tile([128, NCH, C_Z], BF16)
            mso_t = wk1.tile([128, NCH], F32, tag="mso")
            sso_t = wk1.tile([128, NCH], F32, tag="sso")
            LAG_H = 3           # reduce trails TT by this many half-nodes
            NT = 2 * NN         # total half-nodes
            rbf_cur = None
            prods = {}          # t -> prod half tile
            upd_cur = {}        # chunk -> psum tile

            def issue_front(t):
                nonlocal rbf_cur
                n, h = t // 2, t % 2
                if h == 0 and n % 2 == 0:
                    q = n // 2
                    dbc = wk3.tile([128, K * K], F32, tag="dbc")
                    srcD = bc(dD_t[2 * q:2 * q + 2, :], 1, 64)   # [2, 64bc, 1024]
                    nc.sync.dma_start(dbc[:, :], srcD)
                    rbf_cur = wk3.tile([128, K * K], BF16, tag="rbf", name=f"rbf_{t}")
                    if ERF:
                        # erf'(x) = (2/sqrt(pi)) e^{-x^2}; 2/sqrt(pi) folded into wdp2
                        nc.scalar.activation(rbf_cur[:, :], dbc[:, :], AF.Derivative_Erf,
                                             bias=negmu_t[:, :], scale=1.0 / SIGMA)
                    else:
                        sq_t = wk3.tile([128, K * K], BF16, tag="sqt", name=f"sqt_{t}")
                        nc.scalar.activation(sq_t[:, :], dbc[:, :], AF.Square,
                                             bias=negmu_t[:, :], scale=1.0 / SIGMA)
                        nc.scalar.activation(rbf_cur[:, :], sq_t[:, :], AF.Exp, scale=-1.0)
                u = n % 2
                d3_ps = ps_d3.tile([128, 4, 128], F32, tag="d3", name=f"d3_{t}")   # 1 bank
                for bb in range(4):
                    b = 4 * h + bb
                    nc.tensor.matmul(d3_ps[:, bb, :],
                                     rbf_cur[ts(u, 64), ts(b, 128)],
                                     wdp2_t[ts(u, 64), :],
                                     start=True, stop=True)
                prod = wkp.tile([128, 4, 128], BF16, tag="prod", name=f"prod_{t}")
                eesl = bc(ee4_node(n), 1, 4)
                eng = nc.vector  # HW: gpsimd cannot access PSUM (d3_ps)
                eng.tensor_tensor(prod[:, :, :], d3_ps[:, :, :], eesl, op=mult)
                prods[t] = prod

            def issue_back(t):
                n, h = t // 2, t % 2
                g, c = n % 4, n // 4
                if g == 0 and h == 0:
                    upd_cur[c] = ps_u.tile([128, C_Z], F32, tag="u", name=f"upd_{c}")
                upd_ps = upd_cur[c]
                prod = prods.pop(t)
                for bb in range(4):
                    b = 4 * h + bb
                    s0 = 124 - (32 * g + 4 * b)
                    nc.tensor.matmul(upd_ps[:, :], selbig_t[:, s0:s0 + 128],
                                     prod[:, bb, :],
                                     start=(g == 0 and b == 0), stop=False)
                if g == 3 and h == 1:
                    nc.tensor.matmul(upd_ps[:, :], sel4_t[0:4, :],
                                     bias_stage[c][0:4, :], start=False, stop=True)
                    nc.scalar.activation(upd_sb[:, c, :], upd_ps[:, :], AF.Identity,
                                         accum_out=mso_t[:, c:c + 1])
                    scr2 = wk.tile([128, C_Z], BF16, tag="scr2", name=f"scr2_{c}")
                    nc.scalar.activation(scr2[:, :], upd_sb[:, c, :], AF.Square,
                                         accum_out=sso_t[:, c:c + 1])
                    del upd_cur[c]

            # LN_o stats tiles (computed in two halves to overlap the tail with S7)
            mmo_t = wk1.tile([128, NCH], F32, tag="lnso")
            ex2o_t = wk1.tile([128, NCH], F32, tag="lnso2")
            varo_t = wk1.tile([128, NCH], F32, tag="lnso3")
            rso_t = wk1.tile([128, NCH], F32, tag="lnso5")
            mrso_t = wk1.tile([128, NCH], F32, tag="lnso6")
            xoT_t = big.tile([C_Z, NK], BF16)
            outT_t = big.tile([C_Z, NK], BF16)

            def emit_stats(hf):
                sl = slice(8 * hf, 8 * hf + 8)
                nc.vector.tensor_scalar_mul(mmo_t[:, sl], mso_t[:, sl], 1.0 / C_Z)
                nc.vector.tensor_scalar_mul(ex2o_t[:, sl], sso_t[:, sl], 1.0 / C_Z)
                nc.vector.tensor_tensor(varo_t[:, sl], mmo_t[:, sl], mmo_t[:, sl], op=mult)
                nc.vector.tensor_tensor(varo_t[:, sl], ex2o_t[:, sl], varo_t[:, sl], op=sub_op)
                nc.vector.tensor_scalar_add(varo_t[:, sl], varo_t[:, sl], EPS_LN)
                invo = wk.tile([128, 8], F32, tag="invo", name=f"invo_{hf}")
                nc.vector.reciprocal(invo[:, :], varo_t[:, sl])
                nc.scalar.sqrt(rso_t[:, sl], invo[:, :])
                nc.vector.tensor_tensor(mrso_t[:, sl], mmo_t[:, sl], rso_t[:, sl], op=mult)

            def emit_tail(g4):
                xo_ps = ps.tile([128, 4, 128], BF16, tag="m", name=f"xops_{g4}")
                for j in range(4):
                    c = 4 * g4 + j
                    xo = wk.tile([128, C_Z], BF16, tag="xo", name=f"xo_{c}")
                    nc.vector.tensor_scalar(xo[:, :], upd_sb[:, c, :],
                                            rso_t[:, c:c + 1], mrso_t[:, c:c + 1],
                                            op0=mult, op1=sub_op)
                    nc.tensor.transpose(xo_ps[:, j, :], xo[:, :], idb_t[:, :])
                nc.scalar.copy(xoT_t[:, ts(g4, 512)],
                               xo_ps[:, :, :].rearrange("p a b -> p (a b)"))
                f_ps = ps.tile([128, 512], F32, tag="m", name=f"f_{g4}")
                nc.tensor.matmul(f_ps[:, :], wglo_t[:, :], xoT_t[:, ts(g4, 512)],
                                 start=True, stop=True)
                fo = wk.tile([128, 512], BF16, tag="fo", name=f"fo_{g4}")
                nc.scalar.activation(fo[:, :], f_ps[:, :], AF.Identity, bias=blo2_t[:, :])
                nc.vector.tensor_tensor(outT_t[:, ts(g4, 512)], fo[:, :],
                                        ogs_t[:, ts(g4, 512)], op=mult)
                op_ps = ps.tile([128, 4, 128], BF16, tag="m", name=f"ob_{g4}")
                for j in range(4):
                    c = 4 * g4 + j
                    nc.tensor.transpose(op_ps[:, j, :], outT_t[:, ts(c, 128)], idb_t[:, :])
                orow = wk.tile([128, 4, 128], BF16, tag="orow", name=f"or_{g4}")
                nc.vector.tensor_copy(orow[:, :, :], op_ps[:, :, :])
                dst_ap = AP(out_d.ap().tensor, g4 * 512 * C_Z,
                            [[C_Z, 128], [128 * C_Z, 4], [1, C_Z]])
                nc.gpsimd.dma_start(dst_ap, orow[:, :, :])

            for t in range(NT + LAG_H):
                if t < NT:
                    issue_front(t)
                if t >= LAG_H:
                    issue_back(t - LAG_H)
                    if t - LAG_H == 127:
                        emit_stats(0)
                        emit_stats(1)
                        for g4 in range(4):
                            emit_tail(g4)
                    elif t - LAG_H == 191:
                        emit_stats(2)
                        emit_tail(4)
                        emit_tail(5)
            emit_stats(3)
            for g4 in range(6, 8):
                emit_tail(g4)

    nc.compile()
    return nc


def host_prep(inputs):
    """Build per-core input maps from full inputs (host-side sharding + param prep)."""
    nf = np.asarray(inputs["node_features"], np.float32)
    nt = np.asarray(inputs["node_trans"], np.float32)
    ef = np.asarray(inputs["edge_features"], np.float32)
    ei = np.asarray(inputs["edge_index"])
    dst = np.asarray(ei[1], np.int64).reshape(N, K)

    ln_g, ln_b = np.asarray(inputs["ln_g"], np.float32), np.asarray(inputs["ln_b"], np.float32)
    lno_g, lno_b = np.asarray(inputs["lno_g"], np.float32), np.asarray(inputs["lno_b"], np.float32)
    W_eg, b_eg = np.asarray(inputs["W_eg"], np.float32), np.asarray(inputs["b_eg"], np.float32)
    W_ep, b_ep = np.asarray(inputs["W_ep"], np.float32), np.asarray(inputs["b_ep"], np.float32)
    W_og, b_og = np.asarray(inputs["W_og"], np.float32), np.asarray(inputs["b_og"], np.float32)
    W_lo, b_lo = np.asarray(inputs["W_lo"], np.float32), np.asarray(inputs["b_lo"], np.float32)

    def fold(W, b):
        return (ln_g[:, None] * W).astype(np.float32), (ln_b @ W + b).astype(np.float32)
    wgeg, beg = fold(W_eg, b_eg)
    wgep, bep = fold(W_ep, b_ep)
    wgog, bog = fold(W_og, b_og)
    wglo = (lno_g[:, None] * W_lo).astype(np.float32)
    blo2 = (lno_b @ W_lo + b_lo).astype(np.float32)

    nfTa = np.concatenate([nf.T, np.ones((1, N), np.float32)], axis=0)
    wnlra = np.concatenate(
        [np.concatenate([inputs["W_nl"], inputs["W_nr"]], axis=1),
         np.concatenate([inputs["b_nl"], inputs["b_nr"]])[None, :]], axis=0
    ).astype(np.float32)

    ntabT = np.zeros((4, N), np.float32)
    ntabT[0:3] = nt.T

    negmu = np.concatenate([-(MU / SIGMA), -(MU / SIGMA)]).astype(np.float32)

    selbig = np.zeros((128, 252), np.float32)
    for p in range(128):
        selbig[p, 124 + p // 32] = 1.0
    sel4 = np.zeros((4, 128), np.float32)
    for eo in range(128):
        sel4[eo // 32, eo] = 1.0

    shared = dict(
        ntabT=ntabT, wnlra=wnlra,
        wdg=np.asarray(inputs["W_dg"], np.float32), bdg=np.asarray(inputs["b_dg"], np.float32),
        negmu=negmu,
        wdp2=np.concatenate([np.asarray(inputs["W_dp"], np.float32)] * 2, axis=0)
        * (np.sqrt(np.pi) / 2.0 if ERF else 1.0),
        bdp=np.asarray(inputs["b_dp"], np.float32),
        wgeg=wgeg, wgep=wgep, wgog=wgog, beg=beg, bep=bep, bog=bog,
        wglo=wglo, blo2=blo2,
        idf=np.eye(128, dtype=np.float32), idb=np.eye(128, dtype=np.float32),
        selbig=selbig, sel4=sel4,
    )
    import jax.numpy as jnp
    for k in ("wdp2", "wgeg", "wgep", "wgog", "wglo", "idb", "selbig", "sel4"):
        shared[k] = np.asarray(jnp.asarray(shared[k], jnp.bfloat16))

    in_maps = []
    for cc in range(NCORES):
        nsl = slice(cc * NN, (cc + 1) * NN)
        esl = slice(cc * NK, (cc + 1) * NK)
        dstc = dst[nsl].reshape(-1)                       # [4096]
        dstw = np.zeros((128, K), np.int16)
        for g in range(8):
            blk = dstc[512 * g: 512 * (g + 1)]
            dstw[16 * g:16 * (g + 1), :] = blk.reshape(32, 16).T
        m = dict(shared)
        m["ef"] = ef[esl]
        m["dstw"] = dstw
        m["nfTa"] = np.ascontiguousarray(nfTa[:, nsl])
        in_maps.append(m)
    return in_maps


_NC_CACHE = {}


def kernel(**inputs) -> np.ndarray:
    if "nc" not in _NC_CACHE:
        _NC_CACHE["nc"] = build_nc()
    nc = _NC_CACHE["nc"]
    in_maps = host_prep(inputs)
    res = run_bass_kernel_spmd(nc, in_maps, list(range(NCORES)))
    out = np.concatenate([res.results[cc]["out"] for cc in range(NCORES)], axis=0)
    return out.astype(np.float32)


# revision 6
# speedup vs baseline: 1.0463x; 1.0463x over previous
"""Trainium2 Bass kernel V3 for nested triangle multiplicative update.

Data-parallel over nodes N=1024 across 8 cores (128 nodes/core). Per-core:
- rbf exponent via DMA partition-broadcast of D + ACT Square + ACT Exp
  (no PE polynomial, no fp32 matmul).
- d3 computed transposed per node: [128 pair-partitions, z] via matmuls with
  the rbf block as stationary and W_dp as moving.
- triangle j-contraction done on the PE: shifted-selector stationary matmuls
  accumulate 4 nodes x 8 blocks (+ the b_dp*sum_j(ee) bias rank-1 term) into
  one [128 edge, 128 z] PSUM tile, replacing the DVE TensorReduce.
- elementwise d3*ee runs split across DVE and GpSimd.
- LN over z happens in [edge, z] orientation (free-dim stats).
"""
import sys

sys.path.insert(0, "/opt/trn_rl_repo")

import numpy as np

import concourse.bacc as bacc
import concourse.bass as bass
import concourse.mybir as mybir
import concourse.tile as tile
from concourse.bass import AP
from concourse.bass_utils import run_bass_kernel_spmd

F32 = mybir.dt.float32
BF16 = mybir.dt.bfloat16
I16 = mybir.dt.int16

N, K, C_S, C_Z, C_G, R = 1024, 32, 384, 128, 16, 64
NCORES = 8
NN = N // NCORES          # nodes per core = 128
NK = NN * K               # edges per core = 4096
NCH = NK // 128           # 128-row chunks of edges = 32
D_MAX, EPS_LN = 20.0, 1e-5
SIGMA = D_MAX / R                     # 0.3125
MU = np.linspace(0.0, D_MAX, R)      # spacing 20/63

# fraction of q-iterations whose TT runs on gpsimd (tuned from timeline)
POOL_EVERY = 2   # n % POOL_EVERY == 1 -> gpsimd
ERF = True       # single-pass rbf via Derivative_Erf (else Square+Exp)


def ts(i, n):
    return slice(i * n, (i + 1) * n)


def bc(ap, pos, rep):
    """Insert a broadcast (step-0) dim of length rep at free-dim position pos."""
    newap = list(ap.ap)
    newap.insert(pos, [0, rep])
    return AP(ap.tensor, ap.offset, newap)


def build_nc():
    nc = bacc.Bacc("TRN2", target_bir_lowering=False, debug=False)
    P = lambda name, shape, dt: nc.declare_dram_parameter(name, list(shape), dt, isOutput=False)

    ef_d = P("ef", [NK, C_Z], F32)
    dstw_d = P("dstw", [128, K], I16)
    ntabT_d = P("ntabT", [4, N], F32)
    nfTa_d = P("nfTa", [C_S + 1, NN], F32)
    wnlra_d = P("wnlra", [C_S + 1, 2 * C_G], F32)
    wdg_d = P("wdg", [C_G * C_G, C_Z], F32)
    bdg_d = P("bdg", [C_Z], F32)
    negmu_d = P("negmu", [128], F32)
    wdp2_d = P("wdp2", [128, C_Z], BF16)
    bdp_d = P("bdp", [C_Z], F32)
    wgeg_d = P("wgeg", [C_Z, C_Z], BF16)
    wgep_d = P("wgep", [C_Z, C_Z], BF16)
    wgog_d = P("wgog", [C_Z, C_Z], BF16)
    beg_d = P("beg", [C_Z], F32)
    bep_d = P("bep", [C_Z], F32)
    bog_d = P("bog", [C_Z], F32)
    wglo_d = P("wglo", [C_Z, C_Z], BF16)
    blo2_d = P("blo2", [C_Z], F32)
    idf_d = P("idf", [128, 128], F32)
    idb_d = P("idb", [128, 128], BF16)
    selbig_d = P("selbig", [128, 252], BF16)
    sel4_d = P("sel4", [4, 128], BF16)
    out_d = nc.declare_dram_parameter("out", [NK, C_Z], F32, isOutput=True)

    mult, add_op, sub_op = (mybir.AluOpType.mult, mybir.AluOpType.add,
                            mybir.AluOpType.subtract)
    AF = mybir.ActivationFunctionType
    AX = mybir.AxisListType

    with tile.TileContext(nc) as tc:
        with (
            tc.tile_pool(name="big", bufs=1) as big,
            tc.tile_pool(name="wk", bufs=2) as wk,
            tc.tile_pool(name="wk3", bufs=3) as wk3,
            tc.tile_pool(name="wkp", bufs=5) as wkp,
            tc.tile_pool(name="wk1", bufs=1) as wk1,
            tc.tile_pool(name="ps", bufs=2, space="PSUM") as ps,
            tc.tile_pool(name="ps_d3", bufs=4, space="PSUM") as ps_d3,
            tc.tile_pool(name="ps_u", bufs=2, space="PSUM") as ps_u,
        ):
            # ---------- persistent tiles + loads ----------
            ef_t = big.tile([128, NCH, C_Z], F32)
            nc.scalar.dma_start(
                ef_t[:, :, :],
                AP(ef_d.ap().tensor, 0, [[C_Z, 128], [128 * C_Z, NCH], [1, C_Z]]),
            )
            dstw_t = big.tile([128, K], I16)
            nc.sync.dma_start(dstw_t[:], dstw_d.ap()[:, :])
            tabrep_t = big.tile([128, N, 1], F32)
            nc.sync.dma_start(
                tabrep_t[:, :, 0],
                AP(ntabT_d.ap().tensor, 0, [[0, 32], [N, 4], [1, N]]),
            )
            nfTa_t = big.tile([128, 4, NN], F32)
            for c in range(3):
                nc.sync.dma_start(nfTa_t[:, c, :], nfTa_d.ap()[ts(c, 128), :])
            nc.sync.dma_start(nfTa_t[0:1, 3, :], nfTa_d.ap()[384:385, :])
            wnlra_t = big.tile([128, 4, 2 * C_G], F32)
            for c in range(3):
                nc.sync.dma_start(wnlra_t[:, c, :], wnlra_d.ap()[ts(c, 128), :])
            nc.sync.dma_start(wnlra_t[0:1, 3, :], wnlra_d.ap()[384:385, :])
            wdg_t = big.tile([128, 2, C_Z], F32)
            nc.sync.dma_start(wdg_t[:, 0, :], wdg_d.ap()[0:128, :])
            nc.sync.dma_start(wdg_t[:, 1, :], wdg_d.ap()[128:256, :])
            wdp2_t = big.tile([128, C_Z], BF16)
            nc.scalar.dma_start(wdp2_t[:], wdp2_d.ap()[:, :])
            wgeg_t = big.tile([C_Z, C_Z], BF16)
            nc.scalar.dma_start(wgeg_t[:], wgeg_d.ap()[:, :])
            wgep_t = big.tile([C_Z, C_Z], BF16)
            nc.scalar.dma_start(wgep_t[:], wgep_d.ap()[:, :])
            wgog_t = big.tile([C_Z, C_Z], BF16)
            nc.scalar.dma_start(wgog_t[:], wgog_d.ap()[:, :])
            wglo_t = big.tile([C_Z, C_Z], BF16)
            nc.scalar.dma_start(wglo_t[:], wglo_d.ap()[:, :])
            idf_t = big.tile([128, 128], F32)
            nc.scalar.dma_start(idf_t[:], idf_d.ap()[:, :])
            idb_t = big.tile([128, 128], BF16)
            nc.scalar.dma_start(idb_t[:], idb_d.ap()[:, :])
            selbig_t = big.tile([128, 252], BF16)
            nc.scalar.dma_start(selbig_t[:], selbig_d.ap()[:, :])
            sel4_t = big.tile([4, 128], BF16)
            nc.scalar.dma_start(sel4_t[:], sel4_d.ap()[:, :])

            def colvec(name_d):
                t = big.tile([128, 1], F32, tag=f"cv_{name_d.name}")
                nc.scalar.dma_start(t[:, 0:1], AP(name_d.ap().tensor, 0, [[1, 128], [1, 1]]))
                return t
            bdg_t, negmu_t, bdp_t = colvec(bdg_d), colvec(negmu_d), colvec(bdp_d)
            beg_t, bep_t, bog_t, blo2_t = colvec(beg_d), colvec(bep_d), colvec(bog_d), colvec(blo2_d)

            # ---------- S1: gather translations ----------
            gat_t = big.tile([128, 512, 1], F32)
            nc.gpsimd.ap_gather(gat_t[:, :, :], tabrep_t[:, :, :], dstw_t[:, :],
                                channels=128, num_elems=N, d=1, num_idxs=512)
            tn_t = big.tile([128, 4, K], F32)
            for g in range(8):
                for c in range(3):
                    srcg = gat_t[16 * g + c:16 * g + c + 1, :, 0].rearrange(
                        "p (a b) -> p a b", a=16)
                    nc.sync.dma_start(tn_t[16 * g:16 * (g + 1), c, :], srcg)

            # ---------- S2: distances D (two halves of i to bound SBUF) ----------
            dD_t = big.tile([128, K * K], F32)
            for hh in range(4):
                isl = slice(hh * 8, (hh + 1) * 8)
                dif_t = wk.tile([128, 3, 8, K], F32, tag="dif")
                in_j = bc(tn_t[:, 0:3, :], 2, 8)       # [p, c, i(bc8), j]
                in_i = bc(tn_t[:, 0:3, isl], 3, K)     # [p, c, i8, j(bc)]
                nc.gpsimd.tensor_tensor(dif_t[:, :, :, :], in_j, in_i, op=sub_op)
                d3k = dif_t[:, :, :, :].rearrange("p a b c -> p (a b c)")
                nc.gpsimd.tensor_tensor(d3k, d3k, d3k, op=mult)
                d2c_t = wk.tile([128, 8 * K], F32, tag="d2c")
                difap = dif_t[:, :, :, :]
                nc.vector.tensor_reduce(
                    d2c_t[:, :],
                    AP(difap.tensor, difap.offset,
                       [difap.ap[0], [1, 8 * K], [8 * K, 3]]),
                    axis=AX.X, op=add_op)
                nc.vector.tensor_scalar_max(d2c_t[:, :], d2c_t[:, :], 0.0)
                nc.scalar.sqrt(dD_t[:, ts(hh, 256)], d2c_t[:, :])

            # ---------- S4: nl/nr -> outer -> gate3 (g3 stays [z, n]) ----------
            nlnr_ps = ps.tile([128, 2 * C_G], F32, tag="m")
            for c in range(4):
                kdim = 128 if c < 3 else 1
                nc.tensor.matmul(nlnr_ps[:, :], nfTa_t[0:kdim, c, :], wnlra_t[0:kdim, c, :],
                                 start=(c == 0), stop=(c == 3))
            nlnr_t = wk.tile([128, 2 * C_G], F32, tag="nlnr")
            nc.vector.tensor_copy(nlnr_t[:, :], nlnr_ps[:, :])
            outer_t = wk.tile([128, C_G * C_G], F32, tag="outer")
            for a in range(C_G):
                nc.vector.tensor_scalar_mul(outer_t[:, ts(a, C_G)], nlnr_t[:, C_G:2 * C_G],
                                            nlnr_t[:, a:a + 1])
            oT_sb = wk.tile([128, 2, 128], F32, tag="oT")
            for h in range(2):
                oT_ps = ps.tile([128, 128], F32, tag="m")
                nc.tensor.transpose(oT_ps[:, :], outer_t[:, ts(h, 128)], idf_t[:, :])
                nc.vector.tensor_copy(oT_sb[:, h, :], oT_ps[:, :])
            g3_ps = ps.tile([128, 128], F32, tag="m")
            for h in range(2):
                nc.tensor.matmul(g3_ps[:, :], wdg_t[:, h, :], oT_sb[:, h, :],
                                 start=(h == 0), stop=(h == 1))
            g3_t = wk.tile([128, NN], BF16, tag="g3")
            nc.scalar.activation(g3_t[:, :], g3_ps[:, :], AF.Sigmoid, bias=bdg_t[:, :])

            # ---------- S5: LN(ef) stats in [e, z]; xhat -> transpose -> xT ----------
            ms_t = wk.tile([128, NCH], F32, tag="ms")
            nc.vector.tensor_reduce(ms_t[:, :], ef_t[:, :, :], axis=AX.X, op=add_op)
            ss_t = wk.tile([128, NCH], F32, tag="ss")
            for c8 in range(NCH // 8):
                sqg = wk.tile([128, 8, C_Z], BF16, tag="sqg")
                nc.vector.tensor_tensor(sqg[:, :, :], ef_t[:, ts(c8, 8), :],
                                        ef_t[:, ts(c8, 8), :], op=mult)
                nc.vector.tensor_reduce(ss_t[:, ts(c8, 8)], sqg[:, :, :],
                                        axis=AX.X, op=add_op)
            mm_t = wk.tile([128, NCH], F32, tag="lnst")
            nc.vector.tensor_scalar_mul(mm_t[:, :], ms_t[:, :], 1.0 / C_Z)
            ex2_t = wk.tile([128, NCH], F32, tag="lnst2")
            nc.vector.tensor_scalar_mul(ex2_t[:, :], ss_t[:, :], 1.0 / C_Z)
            var_t = wk.tile([128, NCH], F32, tag="lnst3")
            nc.vector.tensor_tensor(var_t[:, :], mm_t[:, :], mm_t[:, :], op=mult)
            nc.vector.tensor_tensor(var_t[:, :], ex2_t[:, :], var_t[:, :], op=sub_op)
            nc.vector.tensor_scalar_add(var_t[:, :], var_t[:, :], EPS_LN)
            inv_t = wk.tile([128, NCH], F32, tag="lnst4")
            nc.vector.reciprocal(inv_t[:, :], var_t[:, :])
            rstd_t = wk.tile([128, NCH], F32, tag="lnst5")
            nc.scalar.sqrt(rstd_t[:, :], inv_t[:, :])
            mrs_t = wk.tile([128, NCH], F32, tag="lnst6")
            nc.vector.tensor_tensor(mrs_t[:, :], mm_t[:, :], rstd_t[:, :], op=mult)

            xT_t = big.tile([C_Z, NK], BF16)
            sge_t = big.tile([C_Z, NK], BF16)
            pep_t = big.tile([C_Z, NK], BF16)
            ogs_t = big.tile([C_Z, NK], BF16)
            ee_t = big.tile([C_Z, NK], BF16)
            eeT_t = big.tile([128, NCH, C_Z], BF16)
            see_t = wk1.tile([128, NN], F32, tag="see")
            sbZ_t = wk1.tile([128, NN], BF16, tag="sbZ")
            bias_stage = {}
            ee4_blk = [big.tile([128, NN // 4, C_Z], BF16, tag=f"ee4_{hb}", name=f"ee4_{hb}")
                       for hb in range(4)]

            for g4 in range(8):
                # xhat for 4 chunks -> transpose -> xT block
                xp_ps = ps.tile([128, 4, 128], BF16, tag="m", name=f"xp_{g4}")
                for j in range(4):
                    c = 4 * g4 + j
                    xh = wk.tile([128, C_Z], BF16, tag="xh", name=f"xh_{c}")
                    nc.gpsimd.tensor_scalar(xh[:, :], ef_t[:, c, :],
                                            rstd_t[:, c:c + 1], mrs_t[:, c:c + 1],
                                            op0=mult, op1=sub_op)
                    nc.tensor.transpose(xp_ps[:, j, :], xh[:, :], idb_t[:, :])
                nc.scalar.copy(xT_t[:, ts(g4, 512)],
                               xp_ps[:, :, :].rearrange("p a b -> p (a b)"))
                # e2 gates for this 512-edge block
                for (w_t, b_t, fn, dst) in ((wgeg_t, beg_t, AF.Sigmoid, sge_t),
                                            (wgep_t, bep_t, AF.Identity, pep_t),
                                            (wgog_t, bog_t, AF.Sigmoid, ogs_t)):
                    e_ps = ps.tile([128, 512], F32, tag="m", name=f"g_{g4}_{fn}")
                    nc.tensor.matmul(e_ps[:, :], w_t[:, :], xT_t[:, ts(g4, 512)],
                                     start=True, stop=True)
                    nc.scalar.activation(dst[:, ts(g4, 512)], e_ps[:, :], fn, bias=b_t[:, :])
                nc.vector.tensor_tensor(ee_t[:, ts(g4, 512)], sge_t[:, ts(g4, 512)],
                                        pep_t[:, ts(g4, 512)], op=mult)
                # fold g3 for nodes [16*g4, 16*g4+16); see partial
                eeap = ee_t[:, :]
                ee3g = AP(eeap.tensor, eeap.offset + 512 * g4, [eeap.ap[0], [K, 16], [1, K]])
                g3bg = bc(g3_t[:, 16 * g4:16 * g4 + 16], 2, K)
                nc.gpsimd.tensor_tensor(ee3g, ee3g, g3bg, op=mult)
                nc.vector.tensor_reduce(see_t[:, 16 * g4:16 * g4 + 16], ee3g,
                                        axis=AX.X, op=add_op)
                nc.vector.tensor_scalar_mul(sbZ_t[:, 16 * g4:16 * g4 + 16],
                                            see_t[:, 16 * g4:16 * g4 + 16], bdp_t[:, :])
                for cc in range(4 * g4, 4 * g4 + 4):
                    sb_ps = ps.tile([4, 128], BF16, tag="m", name=f"sbps_{cc}")
                    nc.tensor.transpose(sb_ps[:, :], sbZ_t[:, 4 * cc:4 * cc + 4],
                                        idb_t[:, :])
                    bst = wk1.tile([4, 128], BF16, tag=f"bst_{cc}", name=f"bst_{cc}")
                    nc.scalar.copy(bst[:, :], sb_ps[:, :])
                    bias_stage[cc] = bst
                # eeT for these 4 chunks
                ep_ps = ps.tile([128, 4, 128], BF16, tag="m", name=f"ep_{g4}")
                for j in range(4):
                    c = 4 * g4 + j
                    nc.tensor.transpose(ep_ps[:, j, :], ee_t[:, ts(c, 128)], idb_t[:, :])
                nc.scalar.copy(eeT_t[:, ts(g4, 4), :].rearrange("p a b -> p (a b)"),
                               ep_ps[:, :, :].rearrange("p a b -> p (a b)"))
                # replicate into ee4 once each 64-node half is ready
                if g4 in (1, 3, 5, 7):
                    hb = g4 // 2
                    for ip in range(4):
                        for n4 in range(4):
                            base = ee4_blk[hb][ts(ip, 32), :, :]
                            dst = AP(base.tensor, base.offset + n4 * C_Z,
                                     [list(base.ap[0]), [4 * C_Z, 8], [1, C_Z]])
                            srce = eeT_t[32 * n4:32 * n4 + 32, 8 * hb:8 * hb + 8, :]
                            nc.sync.dma_start(dst, srce)

            def ee4_node(n):
                return ee4_blk[n // 32][:, n % 32, :]

            # ---------- S7: rbf -> d3T -> prod -> PE-reduce (half-node pipeline) ----------
            upd_sb = big.tile([128, NCH, C_Z], BF16)
            mso_t = wk1.tile([128, NCH], F32, tag="mso")
            sso_t = wk1.tile([128, NCH], F32, tag="sso")
            LAG_H = 3           # reduce trails TT by this many half-nodes
            NT = 2 * NN         # total half-nodes
            rbf_cur = None
            prods = {}          # t -> prod half tile
            upd_cur = {}        # chunk -> psum tile

            def issue_front(t):
                nonlocal rbf_cur
                n, h = t // 2, t % 2
                if h == 0 and n % 2 == 0:
                    q = n // 2
                    dbc = wk3.tile([128, K * K], F32, tag="dbc")
                    srcD = bc(dD_t[2 * q:2 * q + 2, :], 1, 64)   # [2, 64bc, 1024]
                    nc.sync.dma_start(dbc[:, :], srcD)
                    rbf_cur = wk3.tile([128, K * K], BF16, tag="rbf", name=f"rbf_{t}")
                    if ERF:
                        # erf'(x) = (2/sqrt(pi)) e^{-x^2}; 2/sqrt(pi) folded into wdp2
                        nc.scalar.activation(rbf_cur[:, :], dbc[:, :], AF.Derivative_Erf,
                                             bias=negmu_t[:, :], scale=1.0 / SIGMA)
                    else:
                        sq_t = wk3.tile([128, K * K], BF16, tag="sqt", name=f"sqt_{t}")
                        nc.scalar.activation(sq_t[:, :], dbc[:, :], AF.Square,
                                             bias=negmu_t[:, :], scale=1.0 / SIGMA)
                        nc.scalar.activation(rbf_cur[:, :], sq_t[:, :], AF.Exp, scale=-1.0)
                u = n % 2
                d3_ps = ps_d3.tile([128, 4, 128], F32, tag="d3", name=f"d3_{t}")   # 1 bank
                for bb in range(4):
                    b = 4 * h + bb
                    nc.tensor.matmul(d3_ps[:, bb, :],
                                     rbf_cur[ts(u, 64), ts(b, 128)],
                                     wdp2_t[ts(u, 64), :],
                                     start=True, stop=True)
                prod = wkp.tile([128, 4, 128], BF16, tag="prod", name=f"prod_{t}")
                eesl = bc(ee4_node(n), 1, 4)
                eng = nc.vector  # HW: gpsimd cannot access PSUM (d3_ps)
                eng.tensor_tensor(prod[:, :, :], d3_ps[:, :, :], eesl, op=mult)
                prods[t] = prod

            def issue_back(t):
                n, h = t // 2, t % 2
                g, c = n % 4, n // 4
                if g == 0 and h == 0:
                    upd_cur[c] = ps_u.tile([128, C_Z], F32, tag="u", name=f"upd_{c}")
                upd_ps = upd_cur[c]
                prod = prods.pop(t)
                for bb in range(4):
                    b = 4 * h + bb
                    s0 = 124 - (32 * g + 4 * b)
                    nc.tensor.matmul(upd_ps[:, :], selbig_t[:, s0:s0 + 128],
                                     prod[:, bb, :],
                                     start=(g == 0 and b == 0), stop=False)
                if g == 3 and h == 1:
                    nc.tensor.matmul(upd_ps[:, :], sel4_t[0:4, :],
                                     bias_stage[c][0:4, :], start=False, stop=True)
                    nc.scalar.activation(upd_sb[:, c, :], upd_ps[:, :], AF.Identity,
                                         accum_out=mso_t[:, c:c + 1])
                    scr2 = wk.tile([128, C_Z], BF16, tag="scr2", name=f"scr2_{c}")
                    nc.scalar.activation(scr2[:, :], upd_sb[:, c, :], AF.Square,
                                         accum_out=sso_t[:, c:c + 1])
                    del upd_cur[c]

            # LN_o stats tiles (computed in two halves to overlap the tail with S7)
            mmo_t = wk1.tile([128, NCH], F32, tag="lnso")
            ex2o_t = wk1.tile([128, NCH], F32, tag="lnso2")
            varo_t = wk1.tile([128, NCH], F32, tag="lnso3")
            rso_t = wk1.tile([128, NCH], F32, tag="lnso5")
            mrso_t = wk1.tile([128, NCH], F32, tag="lnso6")
            xoT_t = big.tile([C_Z, NK], BF16)
            outT_t = big.tile([C_Z, NK], BF16)

            def emit_stats(hf):
                sl = slice(8 * hf, 8 * hf + 8)
                nc.vector.tensor_scalar_mul(mmo_t[:, sl], mso_t[:, sl], 1.0 / C_Z)
                nc.vector.tensor_scalar_mul(ex2o_t[:, sl], sso_t[:, sl], 1.0 / C_Z)
                nc.vector.tensor_tensor(varo_t[:, sl], mmo_t[:, sl], mmo_t[:, sl], op=mult)
                nc.vector.tensor_tensor(varo_t[:, sl], ex2o_t[:, sl], varo_t[:, sl], op=sub_op)
                nc.vector.tensor_scalar_add(varo_t[:, sl], varo_t[:, sl], EPS_LN)
                invo = wk.tile([128, 8], F32, tag="invo", name=f"invo_{hf}")
                nc.vector.reciprocal(invo[:, :], varo_t[:, sl])
                nc.scalar.sqrt(rso_t[:, sl], invo[:, :])
                nc.vector.tensor_tensor(mrso_t[:, sl], mmo_t[:, sl], rso_t[:, sl], op=mult)

            def emit_tail(g4):
                xo_ps = ps.tile([128, 4, 128], BF16, tag="m", name=f"xops_{g4}")
                for j in range(4):
                    c = 4 * g4 + j
                    xo = wk.tile([128, C_Z], BF16, tag="xo", name=f"xo_{c}")
                    nc.gpsimd.tensor_scalar(xo[:, :], upd_sb[:, c, :],
                                            rso_t[:, c:c + 1], mrso_t[:, c:c + 1],
                                            op0=mult, op1=sub_op)
                    nc.tensor.transpose(xo_ps[:, j, :], xo[:, :], idb_t[:, :])
                nc.scalar.copy(xoT_t[:, ts(g4, 512)],
                               xo_ps[:, :, :].rearrange("p a b -> p (a b)"))
                f_ps = ps.tile([128, 512], F32, tag="m", name=f"f_{g4}")
                nc.tensor.matmul(f_ps[:, :], wglo_t[:, :], xoT_t[:, ts(g4, 512)],
                                 start=True, stop=True)
                fo = wk.tile([128, 512], BF16, tag="fo", name=f"fo_{g4}")
                nc.scalar.activation(fo[:, :], f_ps[:, :], AF.Identity, bias=blo2_t[:, :])
                nc.gpsimd.tensor_tensor(outT_t[:, ts(g4, 512)], fo[:, :],
                                        ogs_t[:, ts(g4, 512)], op=mult)
                op_ps = ps.tile([128, 4, 128], BF16, tag="m", name=f"ob_{g4}")
                for j in range(4):
                    c = 4 * g4 + j
                    nc.tensor.transpose(op_ps[:, j, :], outT_t[:, ts(c, 128)], idb_t[:, :])
                orow = wk.tile([128, 4, 128], BF16, tag="orow", name=f"or_{g4}")
                nc.vector.tensor_copy(orow[:, :, :], op_ps[:, :, :])
                dst_ap = AP(out_d.ap().tensor, g4 * 512 * C_Z,
                            [[C_Z, 128], [128 * C_Z, 4], [1, C_Z]])
                nc.gpsimd.dma_start(dst_ap, orow[:, :, :])

            for t in range(NT + LAG_H):
                if t < NT:
                    issue_front(t)
                if t >= LAG_H:
                    issue_back(t - LAG_H)
                    if t - LAG_H == 127:
                        emit_stats(0)
                        emit_stats(1)
                        for g4 in range(4):
                            emit_tail(g4)
                    elif t - LAG_H == 191:
                        emit_stats(2)
                        emit_tail(4)
                        emit_tail(5)
            emit_stats(3)
            for g4 in range(6, 8):
                emit_tail(g4)

    nc.compile()
    return nc


def host_prep(inputs):
    """Build per-core input maps from full inputs (host-side sharding + param prep)."""
    nf = np.asarray(inputs["node_features"], np.float32)
    nt = np.asarray(inputs["node_trans"], np.float32)
    ef = np.asarray(inputs["edge_features"], np.float32)
    ei = np.asarray(inputs["edge_index"])
    dst = np.asarray(ei[1], np.int64).reshape(N, K)

    ln_g, ln_b = np.asarray(inputs["ln_g"], np.float32), np.asarray(inputs["ln_b"], np.float32)
    lno_g, lno_b = np.asarray(inputs["lno_g"], np.float32), np.asarray(inputs["lno_b"], np.float32)
    W_eg, b_eg = np.asarray(inputs["W_eg"], np.float32), np.asarray(inputs["b_eg"], np.float32)
    W_ep, b_ep = np.asarray(inputs["W_ep"], np.float32), np.asarray(inputs["b_ep"], np.float32)
    W_og, b_og = np.asarray(inputs["W_og"], np.float32), np.asarray(inputs["b_og"], np.float32)
    W_lo, b_lo = np.asarray(inputs["W_lo"], np.float32), np.asarray(inputs["b_lo"], np.float32)

    def fold(W, b):
        return (ln_g[:, None] * W).astype(np.float32), (ln_b @ W + b).astype(np.float32)
    wgeg, beg = fold(W_eg, b_eg)
    wgep, bep = fold(W_ep, b_ep)
    wgog, bog = fold(W_og, b_og)
    wglo = (lno_g[:, None] * W_lo).astype(np.float32)
    blo2 = (lno_b @ W_lo + b_lo).astype(np.float32)

    nfTa = np.concatenate([nf.T, np.ones((1, N), np.float32)], axis=0)
    wnlra = np.concatenate(
        [np.concatenate([inputs["W_nl"], inputs["W_nr"]], axis=1),
         np.concatenate([inputs["b_nl"], inputs["b_nr"]])[None, :]], axis=0
    ).astype(np.float32)

    ntabT = np.zeros((4, N), np.float32)
    ntabT[0:3] = nt.T

    negmu = np.concatenate([-(MU / SIGMA), -(MU / SIGMA)]).astype(np.float32)

    selbig = np.zeros((128, 252), np.float32)
    for p in range(128):
        selbig[p, 124 + p // 32] = 1.0
    sel4 = np.zeros((4, 128), np.float32)
    for eo in range(128):
        sel4[eo // 32, eo] = 1.0

    shared = dict(
        ntabT=ntabT, wnlra=wnlra,
        wdg=np.asarray(inputs["W_dg"], np.float32), bdg=np.asarray(inputs["b_dg"], np.float32),
        negmu=negmu,
        wdp2=np.concatenate([np.asarray(inputs["W_dp"], np.float32)] * 2, axis=0)
        * (np.sqrt(np.pi) / 2.0 if ERF else 1.0),
        bdp=np.asarray(inputs["b_dp"], np.float32),
        wgeg=wgeg, wgep=wgep, wgog=wgog, beg=beg, bep=bep, bog=bog,
        wglo=wglo, blo2=blo2,
        idf=np.eye(128, dtype=np.float32), idb=np.eye(128, dtype=np.float32),
        selbig=selbig, sel4=sel4,
    )
    import jax.numpy as jnp
    for k in ("wdp2", "wgeg", "wgep", "wgog", "wglo", "idb", "selbig", "sel4"):
        shared[k] = np.asarray(jnp.asarray(shared[k], jnp.bfloat16))

    in_maps = []
    for cc in range(NCORES):
        nsl = slice(cc * NN, (cc + 1) * NN)
        esl = slice(cc * NK, (cc + 1) * NK)
        dstc = dst[nsl].reshape(-1)                       # [4096]
        dstw = np.zeros((128, K), np.int16)
        for g in range(8):
            blk = dstc[512 * g: 512 * (g + 1)]
            dstw[16 * g:16 * (g + 1), :] = blk.reshape(32, 16).T
        m = dict(shared)
        m["ef"] = ef[esl]
        m["dstw"] = dstw
        m["nfTa"] = np.ascontiguousarray(nfTa[:, nsl])
        in_maps.append(m)
    return in_maps


_NC_CACHE = {}


def kernel(**inputs) -> np.ndarray:
    if "nc" not in _NC_CACHE:
        _NC_CACHE["nc"] = build_nc()
    nc = _NC_CACHE["nc"]
    in_maps = host_prep(inputs)
    res = run_bass_kernel_spmd(nc, in_maps, list(range(NCORES)))
    out = np.concatenate([res.results[cc]["out"] for cc in range(NCORES)], axis=0)
    return out.astype(np.float32)
